# revision 4
# baseline (speedup 1.0000x reference)
"""Single-head causal attention (B=8, T=2048, D=1024, H=128) on 8 TRN2
NeuronCores — data-parallel over batch (one batch element per core).

Per-core dataflow (all matmul compute in bf16, f32 accumulation):
  1. x [T, D] DMA'd naturally, cast bf16, transposed on TensorE (128x128
     tiles with identity) into xT [d-part, d-tile, t] — d on partitions.
  2. Projections: qT[h, T], kT[h, T] (chunks of 512), v natural [t, h]
     per 128-row tile, with a ones column appended -> v_aug [t, 129].
  3. Scores TRANSPOSED: ST[k-tile 128, q 512] = kT_tile^T @ qT_chunk.
     exp(scale*ST) on ScalarE writes PT bf16 — already in the lhsT
     orientation PV needs, so no P transposes. Causal masking via 4
     precomputed additive masks on diagonal-band tiles (exp -> 0).
  4. O[q 128, 129] += PT_slice^T @ v_aug_tile accumulated over k tiles in
     PSUM; col 128 is the softmax denominator (ones column). Divide and
     DMA out.
"""

import numpy as np

import concourse.bass as bass
import concourse.bacc as bacc
import concourse.mybir as mybir
import concourse.tile as tile
from concourse import bass_utils
from concourse.masks import make_identity

B, T, D, H = 8, 2048, 1024, 128
P = 128
DT = D // P  # 8 d tiles
TT = T // P  # 16 t tiles
CH = 512  # q chunk width
QC = T // CH  # 4 q chunks
N_CORES = 8
SCALE = float(1.0 / np.sqrt(H))
NEG = -1.0e30

F32 = mybir.dt.float32
BF16 = mybir.dt.bfloat16


def build_nc():
    nc = bacc.Bacc("TRN2", target_bir_lowering=False, debug=False)
    x = nc.dram_tensor("x", [T, D], F32, kind="ExternalInput").ap()
    wq_d = nc.dram_tensor("wq", [D, H], F32, kind="ExternalInput").ap()
    wk_d = nc.dram_tensor("wk", [D, H], F32, kind="ExternalInput").ap()
    wv_d = nc.dram_tensor("wv", [D, H], F32, kind="ExternalInput").ap()
    out = nc.dram_tensor("out", [T, H], F32, kind="ExternalOutput").ap()

    with tile.TileContext(nc) as tc:
        _build_body(nc, tc, x, wq_d, wk_d, wv_d, out)
    nc.compile()
    return nc


def _build_body(nc, tc, x, wq_d, wk_d, wv_d, out):
    with (
        tc.tile_pool(name="persist", bufs=1) as persist,
        tc.tile_pool(name="work", bufs=3) as work,
    ):
        # ---- constants ----
        ident = persist.tile([P, P], BF16, tag="ident", name="ident")
        make_identity(nc, ident)

        # 4 additive causal masks for the diagonal band, j = k_tile - 4*c.
        # keep (0.0) iff qq - kk - j*128 >= 0 else NEG
        masks = persist.tile([P, 4, CH], F32, tag="masks", name="masks")
        nc.gpsimd.memset(masks[:], 0.0)
        for j in range(4):
            nc.gpsimd.affine_select(
                out=masks[:, j, :],
                in_=masks[:, j, :],
                compare_op=mybir.AluOpType.is_ge,
                fill=NEG,
                base=-(j * P),
                pattern=[[1, CH]],
                channel_multiplier=-1,
            )

        # ---- weights: [D, H] -> [p, dt, h] then cast bf16 ----
        w_bf = []
        for nm, wd in (("wq", wq_d), ("wk", wk_d), ("wv", wv_d)):
            wf = work.tile([P, DT, H], F32, tag="wf32", name=f"{nm}_f32")
            nc.sync.dma_start(wf[:], wd.rearrange("(a p) h -> p a h", p=P))
            wb = persist.tile([P, DT, H], BF16, tag=f"{nm}_bf", name=f"{nm}_bf")
            nc.vector.tensor_copy(wb[:], wf[:])
            w_bf.append(wb)
        wq_bf, wk_bf, wv_bf = w_bf

        # ---- persistent activations ----
        xT = persist.tile([P, DT, T], BF16, tag="xT", name="xT")
        qT = persist.tile([P, T], BF16, tag="qT", name="qT")
        kT = persist.tile([P, T], BF16, tag="kT", name="kT")
        v_aug = persist.tile([P, TT, H + 1], BF16, tag="v_aug", name="v_aug")
        nc.vector.memset(v_aug[:], 1.0)  # col H stays 1.0 (ones trick)

        # ---- phase 1: transpose x, projections ----
        with tc.tile_pool(name="ps1", bufs=2, space="PSUM") as ps1:
            for c in range(QC):
                for tt in range(4 * c, 4 * c + 4):
                    x_nat = work.tile([P, D], F32, tag="x_nat", name=f"x_nat{tt}")
                    nc.sync.dma_start(x_nat[:], x[tt * P : (tt + 1) * P, :])
                    x_bf = work.tile([P, D], BF16, tag="x_bf", name=f"x_bf{tt}")
                    nc.vector.tensor_copy(x_bf[:], x_nat[:])
                    for half in range(2):
                        tr_ps = ps1.tile(
                            [P, 4 * P], BF16, tag="tr", name=f"tr{tt}_{half}"
                        )
                        for j in range(4):
                            dt = half * 4 + j
                            nc.tensor.transpose(
                                tr_ps[:, j * P : (j + 1) * P],
                                x_bf[:, dt * P : (dt + 1) * P],
                                ident,
                            )
                        # one strided copy: psum [128, 4*128] -> xT[:, 4 dts, t-tile]
                        dst = xT[:, half * 4 : half * 4 + 4, tt * P : (tt + 1) * P]
                        src = tr_ps.rearrange("p (a t) -> p a t", a=4)
                        if (tt + half) % 2 == 0:
                            nc.vector.tensor_copy(dst, src)
                        else:
                            nc.scalar.copy(dst, src)
                # qT / kT for this chunk of t
                for nm, wb, dstT in (("q", wq_bf, qT), ("k", wk_bf, kT)):
                    pr_ps = ps1.tile([P, CH], F32, tag="proj", name=f"{nm}T_ps{c}")
                    for dt in range(DT):
                        nc.tensor.matmul(
                            pr_ps[:],
                            wb[:, dt, :],
                            xT[:, dt, c * CH : (c + 1) * CH],
                            start=(dt == 0),
                            stop=(dt == DT - 1),
                        )
                    if nm == "q":
                        nc.vector.tensor_copy(dstT[:, c * CH : (c + 1) * CH], pr_ps[:])
                    else:
                        nc.scalar.copy(dstT[:, c * CH : (c + 1) * CH], pr_ps[:])
                # v natural for the 4 t-tiles of this chunk
                for tt in range(4 * c, 4 * c + 4):
                    v_ps = ps1.tile([P, H], F32, tag="vproj", name=f"v_ps{tt}")
                    for dt in range(DT):
                        nc.tensor.matmul(
                            v_ps[:],
                            xT[:, dt, tt * P : (tt + 1) * P],
                            wv_bf[:, dt, :],
                            start=(dt == 0),
                            stop=(dt == DT - 1),
                        )
                    nc.vector.tensor_copy(v_aug[:, tt, 0:H], v_ps[:])

        # ---- phase 2: attention main loop ----
        with (
            tc.tile_pool(name="ps_st", bufs=2, space="PSUM") as ps_st,
            tc.tile_pool(name="ps_o", bufs=4, space="PSUM") as ps_o,
        ):
            for c in range(QC):
                last = 4 * c + 3
                o_ps = [
                    ps_o.tile([P, H + 1], F32, tag="o", name=f"o{c}_{s}")
                    for s in range(4)
                ]
                st_ps = {}

                def emit_s(i, c=c, st_ps=st_ps):
                    st = ps_st.tile([P, CH], F32, tag="st", name=f"st{c}_{i}")
                    nc.tensor.matmul(
                        st[:],
                        kT[:, i * P : (i + 1) * P],
                        qT[:, c * CH : (c + 1) * CH],
                        start=True,
                        stop=True,
                    )
                    st_ps[i] = st

                emit_s(0)
                for i in range(last + 1):
                    if i < last:
                        emit_s(i + 1)  # keep PE busy while ACT does exp(i)
                    st = st_ps.pop(i)
                    j = i - 4 * c
                    if j >= 0:
                        nc.vector.tensor_add(st[:], st[:], masks[:, j, :])
                    pt = work.tile([P, CH], BF16, tag="pt", name=f"pt{c}_{i}")
                    nc.scalar.activation(
                        pt[:], st[:], mybir.ActivationFunctionType.Exp, scale=SCALE
                    )
                    for s in range(4):
                        if i <= 4 * c + s:
                            nc.tensor.matmul(
                                o_ps[s][:],
                                pt[:, s * P : (s + 1) * P],
                                v_aug[:, i, :],
                                start=(i == 0),
                                stop=(i == 4 * c + s),
                            )
                for s in range(4):
                    qt_idx = 4 * c + s
                    recip = work.tile([P, 1], F32, tag="recip", name=f"rcp{qt_idx}")
                    nc.vector.reciprocal(recip[:], o_ps[s][:, H : H + 1])
                    o_sb = work.tile([P, H], F32, tag="o_sb", name=f"o_sb{qt_idx}")
                    nc.vector.tensor_scalar_mul(o_sb[:], o_ps[s][:, 0:H], recip[:])
                    nc.sync.dma_start(
                        out[qt_idx * P : (qt_idx + 1) * P, :], o_sb[:]
                    )


_NC_CACHE = None


def _get_nc():
    global _NC_CACHE
    if _NC_CACHE is None:
        _NC_CACHE = build_nc()
    return _NC_CACHE


def kernel(**inputs):
    x = np.ascontiguousarray(np.asarray(inputs["x"], dtype=np.float32))
    wq = np.ascontiguousarray(np.asarray(inputs["Wq"], dtype=np.float32))
    wk = np.ascontiguousarray(np.asarray(inputs["Wk"], dtype=np.float32))
    wv = np.ascontiguousarray(np.asarray(inputs["Wv"], dtype=np.float32))
    assert x.shape == (B, T, D)
    nc = _get_nc()
    in_maps = [
        {"x": np.ascontiguousarray(x[b]), "wq": wq, "wk": wk, "wv": wv}
        for b in range(N_CORES)
    ]
    res = bass_utils.run_bass_kernel_spmd(nc, in_maps, core_ids=list(range(N_CORES)))
    return np.stack([res.results[b]["out"] for b in range(N_CORES)], axis=0)
